# revision 6
# baseline (speedup 1.0000x reference)
"""Trainium2 Bass kernel for nn_DynamicSelectiveHyperNet.

Strategy
--------
Shard the target-parameter axis T across the 8 NeuronCores (no collectives;
the gated head-sum is computed locally per T-slice). Each core runs all 8
heads for its slice. Streams are fp8-compressed (att: 64 feat rows + rank-4
embeds/bias fold, x128; gen: 4 column groups folded into K=128).

v3 structure (PE-column + LDWEIGHTS + sync-latency aware):
  * att matmuls in fp8 DoubleRow perf mode; the two k-subtiles carry two
    adjacent 512-wide column groups via a block stationary, so 2 DR matmuls
    per (head, supertile) replace 4 plain fp8 matmuls.
  * gen_b2 bias: host-regrouped [H, 4, ts/4] stream + one K=4 matmul
    (block stationary from the gate row) instead of 4 K=1 matmuls.
  * head-sum on PE via identity-matmul PSUM accumulation, delayed by one
    full supertile so PE never stalls on the ACT(sigmoid)->DVE(product)
    chain; the 8 idsums run back-to-back (one shared stationary).
  * DMAs are batched: 1 bf16 const blob, 1 f32 blob, 1 ohdr, 4 att chunk
    DMAs (all heads per chunk), 1 gen, 1 genb, 2 out = ~11 DMAs/iteration.
  * preamble flattened: one matmul for all heads' hmid; lgen2 built via
    DVE transposes/copies (no PSUM, no per-head PE round trips).

The preamble runs inside the repeat loop used for timing, so amortized
per-iteration numbers include it.
"""

import sys

sys.path.insert(0, "/opt/trn_rl_repo")

import json

import numpy as np

import concourse.bass as bass
import concourse.bass2jax as _bass2jax
import concourse.bass_utils as _bass_utils
import concourse.tile as tile
from concourse import mybir
from concourse.bass_utils import run_bass_kernel_spmd

AF = mybir.ActivationFunctionType
ALU = mybir.AluOpType
F32 = mybir.dt.float32
BF16 = mybir.dt.bfloat16
F8 = mybir.dt.float8e4
AX = mybir.AxisListType
DR = mybir.MatmulPerfMode.DoubleRow

B = 8
H = 8
NP = 4          # target param groups
FEAT = 64
EMB = 32
HIN = 96        # FEAT + EMB
GH = 32         # generator hidden
T = 101770
NCORES = 8
TS = 12800      # per-core T shard (8*TS = 102400 >= T, zero padded)
SUP = 2048      # supertile columns (4 col-groups x 512)
NSUB = 512
KFE = 896       # 784 padded to 7*128
PB = NP * B     # 32
KA = FEAT + NP  # 68: att stream rows (feats part + rank-4 embeds/bias fold)
N_SUP = 6       # full supertiles per core; plus one 512-wide tail
CHSUP = 2       # att DMA chunk size in supertiles

SC_W = 128.0    # host scale on att stream values
SC_F = 16.0     # device scale on feats in the att stationary
SC_OH = 16.0    # onehot value (matches SC_F so F rows align with A1 rows)
STAGGERED = True

# const blob column offsets (bf16 blob [128, CB_TOT])
CB_FE1 = 0
CB_XT = CB_FE1 + 896
CB_FW2 = CB_XT + 56
CB_GWT = CB_FW2 + 64
CB_SEL4 = CB_GWT + 8
CB_G1 = CB_SEL4 + 32
CB_ID = CB_G1 + 256
CB_EMB = CB_ID + 128
CB_TOT = CB_EMB + 32

# ---------------------------------------------------------------------------
# Workaround: this container's walrus build rejects more than one sync-wait
# command per instruction, while Tile freely attaches several. Split the
# extra waits onto same-engine NoOps inserted just before the instruction.
# ---------------------------------------------------------------------------
_orig_compile_bir_kernel = _bass_utils.compile_bir_kernel


def _split_multi_waits(bir):
    for fn in bir.get("functions", []):
        for bb in fn.get("blocks", []):
            out = []
            for ins in bb.get("instructions", []):
                si = ins.get("sync_info")
                waits = (si or {}).get("on_wait") or []
                if len(waits) > 1:
                    for k, w in enumerate(waits[:-1]):
                        out.append({
                            "debug": ins.get("debug", 0),
                            "engine": ins["engine"],
                            "ins": [],
                            "name": f"{ins['name']}-wsplit{k}",
                            "opcode": "NoOp",
                            "outs": [],
                            "sync_info": {"on_update": [], "on_wait": [w]},
                        })
                    si["on_wait"] = [waits[-1]]
                out.append(ins)
            bb["instructions"] = out
    return bir


def _patched_compile_bir_kernel(bir_json, tmpdir, neff_name="file.neff"):
    bir = _split_multi_waits(json.loads(bir_json))
    return _orig_compile_bir_kernel(json.dumps(bir).encode(), tmpdir,
                                    neff_name=neff_name)


def _install_patch():
    _bass_utils.compile_bir_kernel = _patched_compile_bir_kernel
    _bass2jax.compile_bir_kernel = _patched_compile_bir_kernel


_install_patch()


# ---------------------------------------------------------------------------
# Device program
# ---------------------------------------------------------------------------
def _build_bass(ts=TS, repeats=1):
    nc = bass.Bass()

    att_in = nc.dram_tensor("att_in", [H, KA, ts], F8, kind="ExternalInput")
    gen_in = nc.dram_tensor("gen_in", [H, 4 * GH, ts // 4], F8,
                            kind="ExternalInput")
    genb_in = nc.dram_tensor("genb_in", [H, 4, ts // 4], BF16,
                             kind="ExternalInput")
    cblob = nc.dram_tensor("cblob", [128, CB_TOT], BF16, kind="ExternalInput")
    fblob = nc.dram_tensor("fblob", [128, 2], F32, kind="ExternalInput")
    ohdr = nc.dram_tensor("ohdr", [NP, 512], F8, kind="ExternalInput")
    out = nc.dram_tensor("out", [PB, ts], BF16, kind="ExternalOutput")

    assert ts == N_SUP * SUP + NSUB

    with tile.TileContext(nc) as tc:
        with (
            tc.tile_pool(name="const", bufs=1) as cp,
            tc.tile_pool(name="stream", bufs=1) as sp,
            tc.tile_pool(name="attstream", bufs=2) as ap,
            tc.tile_pool(name="psumA", bufs=3, space="PSUM") as ppA,
            tc.tile_pool(name="psumG", bufs=2, space="PSUM") as ppG,
            tc.tile_pool(name="prepsum", bufs=1, space="PSUM") as prep,
            tc.tile_pool(name="impp", bufs=3) as impp,
            tc.tile_pool(name="tmpp", bufs=16) as tmpp,
            tc.tile_pool(name="accp", bufs=1) as accp,
        ):
            def body():
                _emit_iter(nc, tc, cp, sp, ap, ppA, ppG, prep, impp, tmpp,
                           accp, att_in, gen_in, genb_in, cblob, fblob,
                           ohdr, out)

            if repeats > 1:
                with tc.For_i(0, repeats,
                              staggered_reset=STAGGERED,
                              hint_engines=(mybir.EngineType.PE,
                                            mybir.EngineType.SP,
                                            mybir.EngineType.DVE,
                                            mybir.EngineType.Activation)):
                    body()
            else:
                body()
    return nc


def _emit_iter(nc, tc, cp, sp, ap, ppA, ppG, prep, impp, tmpp, accp,
               att_in, gen_in, genb_in, cblob, fblob, ohdr, out):
    ts = out.shape[1]
    inv_scale = 1.0 / (SC_W * SC_F)
    gsc = 1.0 / SC_W

    # ---- stream DMAs issued first (no preamble dependency) ----------------
    gen8 = sp.tile([4 * GH, H, ts // 4], F8, name="gen8", tag="gen8")
    nc.scalar.dma_start(gen8[:], gen_in.rearrange("h p c -> p h c"))
    genb8 = sp.tile([4, H, ts // 4], BF16, name="genb8", tag="genb8")
    nc.scalar.dma_start(genb8[:], genb_in.rearrange("h p c -> p h c"))

    chunks = [(s0, min(CHSUP, N_SUP + 1 - s0))
              for s0 in range(0, N_SUP + 1, CHSUP)]
    att_tiles = {}

    def issue_att_chunk(ci):
        s0, cnt = chunks[ci]
        cstart = s0 * SUP
        csize = min(CHSUP * SUP, ts - cstart)
        t = ap.tile([KA, H, CHSUP * SUP], F8, name="att8", tag="att8")
        nc.sync.dma_start(t[:, :, :csize],
                          att_in[:, :, cstart:cstart + csize]
                          .rearrange("h p c -> p h c"))
        att_tiles[ci] = t

    issue_att_chunk(0)

    # ---- constant loads (3 DMAs) ------------------------------------------
    blob = cp.tile([128, CB_TOT], BF16, name="blob", tag="blob")
    nc.sync.dma_start(blob[:], cblob[:])
    fb = cp.tile([128, 2], F32, name="fb", tag="fb")
    nc.sync.dma_start(fb[:], fblob[:])

    fe1_t = blob[:, CB_FE1:CB_XT].rearrange("p (o m) -> p o m", o=7)
    xt_t = blob[:, CB_XT:CB_FW2].rearrange("p (o m) -> p o m", o=7)
    fw2_t = blob[:, CB_FW2:CB_GWT]
    gwt_t = blob[:FEAT + 1, CB_GWT:CB_SEL4]
    sel4_t = blob[:B, CB_SEL4:CB_G1]
    g1_t = blob[:HIN + 1, CB_G1:CB_ID]
    id_t = blob[:, CB_ID:CB_EMB]
    emb_t = blob[:EMB, CB_EMB:CB_TOT]

    issue_att_chunk(1)

    hinT = cp.tile([HIN + 1, PB], BF16)     # [97, 32] stationary (gen_W1)
    lgen2 = cp.tile([128, H * 128], BF16, name="lgen2", tag="lgen2")
    S4all = cp.tile([4, H * 128], BF16, name="S4all", tag="S4all")
    nc.vector.memset(S4all[:], 0.0)
    # att DoubleRow stationary: two [68, 2, 128] tiles side by side; subtile
    # i of tile t holds hinF at output block 32*(2t+i)
    attStat = cp.tile([KA, 512], F8, name="attStat", tag="attStat")
    nc.vector.memset(attStat[:], 0.0)

    # ---- feature extractor ------------------------------------------------
    psf = prep.tile([128, 256], F32, tag="pre1")
    for o in range(7):
        nc.tensor.matmul(psf[:, :B], fe1_t[:, o, :], xt_t[:, o, :],
                         start=(o == 0), stop=(o == 6))
    relu1 = cp.tile([128, B], BF16)
    nc.scalar.activation(relu1[:], psf[:, :B], AF.Relu, bias=fb[:, 0:1])

    psf2 = prep.tile([128, 256], F32, tag="pre1")
    nc.tensor.matmul(psf2[:FEAT, :B], fw2_t, relu1[:], start=True, stop=True)
    featsT = cp.tile([FEAT + 1, B], BF16)   # [65, 8], row 64 = ones
    nc.scalar.activation(featsT[:FEAT, :], psf2[:FEAT, :B], AF.Identity,
                         bias=fb[:FEAT, 1:2])
    nc.vector.memset(featsT[FEAT:FEAT + 1, :], 1.0)

    # ---- head gate (softmax over heads, normalization folded) -------------
    psgl = prep.tile([128, 256], F32, tag="pre1")
    nc.tensor.matmul(psgl[:B, :B], featsT[:], gwt_t, start=True, stop=True)
    gateb = cp.tile([32, 32], F32)          # gate[b, h] in [0:8, 0:8]
    nc.vector.memset(gateb[:], 0.0)
    nc.scalar.activation(gateb[:B, :B], psgl[:B, :B], AF.Exp)
    sums = cp.tile([B, 1], F32)
    nc.vector.tensor_reduce(sums[:], gateb[:B, :B], AX.X, ALU.add)
    recip = cp.tile([B, 1], F32)
    nc.vector.reciprocal(recip[:], sums[:])
    nc.vector.tensor_scalar_mul(gateb[:B, :B], gateb[:B, :B], recip[:])
    gatebT = cp.tile([32, 32], F32)         # gate[h, b] in [0:8, 0:8]
    nc.vector.transpose(gatebT[:], gateb[:])
    gatebT_bf = cp.tile([32, 32], BF16)
    nc.vector.tensor_copy(gatebT_bf[:], gatebT[:])
    # gate column per (pb, h): gcols[pb, h] = gate[h, pb % 8]
    psgc = prep.tile([128, 256], F32, tag="pre1")
    nc.tensor.matmul(psgc[:PB, :B], sel4_t, gatebT_bf[:B, :B],
                     start=True, stop=True)
    gcols = cp.tile([PB, B], F32)
    nc.vector.tensor_copy(gcols[:], psgc[:PB, :B])

    # ---- S4all: S4[g, 128h+32g+pb] = gate[h, pb]*SC_W ---------------------
    gcwt = cp.tile([B, PB], BF16, name="gcwt", tag="gcwt")
    for p in range(NP):
        nc.scalar.mul(gcwt[:, p * B:(p + 1) * B], gateb[:B, :B],
                      float(SC_W))
    for g in range(4):
        dst = S4all[g:g + 1, :].rearrange("p (h c) -> p h c", c=128)
        dst = dst[:, :, 32 * g:32 * g + PB]
        nc.sync.dma_start(dst, gcwt[:])

    # ---- hinT (stationary for the gen_W1 matmuls) -------------------------
    for p in range(NP):
        nc.vector.tensor_copy(hinT[:FEAT, p * B:(p + 1) * B],
                              featsT[:FEAT, :])
    nc.vector.tensor_copy(hinT[FEAT:HIN, :], emb_t)
    nc.vector.memset(hinT[HIN:HIN + 1, :], 1.0)

    # ---- attStat: hinF feats block (x SC_F) replicated to 4 positions -----
    hinf_s = cp.tile([FEAT, PB], F8, name="hinf_s", tag="hinf_s")
    for p in range(NP):
        nc.scalar.mul(hinf_s[:, p * B:(p + 1) * B], featsT[:FEAT, :],
                      float(SC_F))
    for t2 in range(2):
        for i in range(2):
            g = 2 * t2 + i
            o = 256 * t2 + 128 * i + 32 * g
            nc.vector.tensor_copy(attStat[:FEAT, o:o + 32], hinf_s[:])
    nc.sync.dma_start(attStat[FEAT:KA, :], ohdr[:])

    # ---- all heads' hmid in one shot; lgen2 via DVE transpose/copies ------
    psh = prep.tile([128, 256], F32, tag="pre1")
    nc.tensor.matmul(psh[:PB, :], hinT[:], g1_t, start=True, stop=True)
    hmid_all = cp.tile([PB, H * GH], F32, name="hmid_all", tag="hmid_all")
    nc.scalar.activation(hmid_all[:], psh[:PB, :], AF.Relu)
    for h in range(H):
        nc.vector.tensor_scalar_mul(hmid_all[:, h * GH:(h + 1) * GH],
                                    hmid_all[:, h * GH:(h + 1) * GH],
                                    gcols[:, h:h + 1])
    hmidT_all = cp.tile([PB, H * GH], F32, name="hmidT_all", tag="hmidT_all")
    for h in range(H):
        nc.vector.transpose(hmidT_all[:, h * GH:(h + 1) * GH],
                            hmid_all[:, h * GH:(h + 1) * GH])
    nc.vector.memset(lgen2[:], 0.0)
    for h in range(H):
        for g in range(4):
            nc.vector.tensor_copy(
                lgen2[32 * g:32 * (g + 1),
                      h * 128 + 32 * g:h * 128 + 32 * (g + 1)],
                hmidT_all[:, h * GH:(h + 1) * GH])

    # ---- main loop: s-outer / h-inner; idsum delayed one supertile --------
    accout = accp.tile([128, ts // 4], BF16, name="accout", tag="accb")

    def emit_idsum(prev_s, prev_tmps):
        pncols = SUP if prev_s < N_SUP else NSUB
        pns = pncols // 4
        psC = ppG.tile([128, NSUB], F32, tag="psC")
        for h in range(H):
            nc.tensor.matmul(psC[:, :pns], id_t, prev_tmps[h][:, :pns],
                             start=(h == 0), stop=(h == H - 1))
        a0 = prev_s * NSUB
        nc.scalar.copy(accout[:, a0:a0 + pns], psC[:, :pns])

    prev = None
    for ci, (s0, cnt) in enumerate(chunks):
        if ci + 2 < len(chunks):
            issue_att_chunk(ci + 2)
        for s in range(s0, s0 + cnt):
            ncols = SUP if s < N_SUP else NSUB
            ns = ncols // 4
            rel = (s - s0) * SUP
            q0 = s * NSUB
            att_t = att_tiles[ci]
            tmps = []
            for h in range(H):
                psA = ppA.tile([128, NSUB], F32, tag="psA")
                psG = ppG.tile([128, NSUB], F32, tag="psG")
                for t2 in range(2):
                    o = rel + 2 * t2 * ns
                    nc.tensor.matmul(
                        psA[:, :ns],
                        attStat[:, 256 * t2:256 * (t2 + 1)]
                        .rearrange("p (i c) -> p i c", i=2),
                        att_t[:, h, o:o + 2 * ns]
                        .rearrange("p (i n) -> p i n", i=2),
                        start=(t2 == 0), stop=(t2 == 1),
                        perf_mode=DR)
                nc.tensor.matmul(psG[:, :ns], lgen2[:, h * 128:(h + 1) * 128],
                                 gen8[:, h, q0:q0 + ns],
                                 start=True, stop=False)
                nc.tensor.matmul(psG[:, :ns], S4all[:, h * 128:(h + 1) * 128],
                                 genb8[:, h, q0:q0 + ns],
                                 start=False, stop=True)
                imp = impp.tile([128, NSUB], F32, tag="imp")
                nc.scalar.activation(imp[:, :ns], psA[:, :ns], AF.Sigmoid,
                                     scale=float(inv_scale))
                tmp = tmpp.tile([128, NSUB], BF16, tag="tmp")
                nc.vector.scalar_tensor_tensor(tmp[:, :ns], psG[:, :ns],
                                               float(gsc), imp[:, :ns],
                                               ALU.mult, ALU.mult)
                tmps.append(tmp)
            if prev is not None:
                emit_idsum(*prev)
            prev = (s, tmps)
    emit_idsum(*prev)

    # ---- output writeback (one DMA per supertile) -------------------------
    for s in range(N_SUP + 1):
        ncols = SUP if s < N_SUP else NSUB
        ns = ncols // 4
        nc.scalar.dma_start(
            out[:, s * SUP:s * SUP + ncols].rearrange("p (g c) -> g p c", g=4),
            accout[:, s * NSUB:s * NSUB + ns])
    return nc


_NC_CACHE = None


def _get_nc():
    global _NC_CACHE
    if _NC_CACHE is None:
        _NC_CACHE = _build_bass()
    return _NC_CACHE


# ---------------------------------------------------------------------------
# Host wrapper
# ---------------------------------------------------------------------------
LAST_RESULTS = None
LAST_IN_MAPS = None


def kernel(x, fe_W1, fe_b1, fe_W2, fe_b2, embeds,
           gen_W1, gen_b1, gen_W2, gen_b2, att_W, att_b,
           gate_W, gate_b):
    f32 = np.float32
    np_f8 = mybir.dt.np(F8)
    np_bf = mybir.dt.np(BF16)
    x = np.asarray(x, f32)
    fe_W1 = np.asarray(fe_W1, f32)
    fe_b1 = np.asarray(fe_b1, f32)
    fe_W2 = np.asarray(fe_W2, f32)
    fe_b2 = np.asarray(fe_b2, f32)
    embeds = np.asarray(embeds, f32)
    gen_W1 = np.asarray(gen_W1, f32)
    gen_b1 = np.asarray(gen_b1, f32)
    gen_W2 = np.asarray(gen_W2, f32)
    gen_b2 = np.asarray(gen_b2, f32)
    att_W = np.asarray(att_W, f32)
    att_b = np.asarray(att_b, f32)
    gate_W = np.asarray(gate_W, f32)
    gate_b = np.asarray(gate_b, f32)

    # --- att stream: [H, 68, T_pad] (feats rows + rank-4 embeds/bias fold) --
    tpad = NCORES * TS
    att_all = np.zeros((H, KA, tpad), np_f8)
    att_all[:, :FEAT, :T] = (att_W[:, :, :FEAT].transpose(0, 2, 1)
                             * SC_W).astype(np_f8)
    F = np.einsum('pj,htj->hpt', embeds, att_W[:, :, FEAT:]) \
        + att_b[:, None, :]
    att_all[:, FEAT:, :T] = (F * SC_W).astype(np_f8)
    # --- gen stream ---------------------------------------------------------
    G8 = np.zeros((H, GH, tpad), np.float32)
    G8[:, :, :T] = gen_W2.transpose(0, 2, 1) * SC_W
    genb_all = np.zeros((H, 1, tpad), np_bf)
    genb_all[:, 0, :T] = gen_b2.astype(np_bf)

    # --- const blobs ---
    xt = np.zeros((KFE, B), f32)
    xt[:784] = x.T
    fe1t = np.zeros((KFE, 128), f32)
    fe1t[:784] = fe_W1.T
    gwt = np.concatenate([gate_W.T, gate_b[None, :]], axis=0)
    emb = np.repeat(embeds.T[:, :, None], B, axis=2).reshape(EMB, PB)
    sel4 = np.tile(np.eye(B, dtype=f32), NP)
    g1in = np.concatenate([gen_W1.transpose(0, 2, 1), gen_b1[:, None, :]],
                          axis=1)                      # [H, 97, 32]
    g1in = g1in.transpose(1, 0, 2).reshape(HIN + 1, H * GH)

    def pad128(a):
        o = np.zeros((128, a.shape[1]), f32)
        o[:a.shape[0]] = a
        return o

    cb = np.zeros((128, CB_TOT), f32)
    cb[:, CB_FE1:CB_XT] = fe1t.reshape(7, 128, 128).transpose(1, 0, 2) \
        .reshape(128, 896)
    cb[:, CB_XT:CB_FW2] = xt.reshape(7, 128, B).transpose(1, 0, 2) \
        .reshape(128, 56)
    cb[:, CB_FW2:CB_GWT] = fe_W2.T
    cb[:, CB_GWT:CB_SEL4] = pad128(gwt)
    cb[:, CB_SEL4:CB_G1] = pad128(sel4)
    cb[:, CB_G1:CB_ID] = pad128(g1in)
    cb[:, CB_ID:CB_EMB] = np.eye(128, dtype=f32)
    cb[:, CB_EMB:CB_TOT] = pad128(emb)

    fblob_a = np.zeros((128, 2), f32)
    fblob_a[:, 0] = fe_b1
    fblob_a[:FEAT, 1] = fe_b2

    # onehot rows for the DoubleRow att stationary: [4, 512]; subtile i of
    # tile t carries the onehot block at columns 256t+128i+32(2t+i)
    oh = np.repeat(np.eye(NP, dtype=f32), B, axis=1) * SC_OH   # [4, 32]
    ohdr_a = np.zeros((NP, 512), f32)
    for t2 in range(2):
        for i in range(2):
            g = 2 * t2 + i
            o = 256 * t2 + 128 * i + 32 * g
            ohdr_a[:, o:o + PB] = oh

    shared = {
        "cblob": np.ascontiguousarray(cb.astype(np_bf)),
        "fblob": fblob_a,
        "ohdr": np.ascontiguousarray(ohdr_a.astype(np_f8)),
    }

    def _regroup(a):
        # [H, k, TS] -> [H, 4k, TS//4]: column group g of each supertile
        # becomes contraction rows [g*k, (g+1)*k)
        k = a.shape[1]
        o = np.zeros((H, 4 * k, TS // 4), a.dtype)
        for s in range(7):
            c0 = s * SUP
            ns = (SUP if s < 6 else NSUB) // 4
            for g in range(4):
                o[:, g * k:(g + 1) * k, s * NSUB:s * NSUB + ns] = \
                    a[:, :, c0 + g * ns:c0 + (g + 1) * ns]
        return o

    in_maps = []
    for c in range(NCORES):
        sl = slice(c * TS, (c + 1) * TS)
        m = dict(shared)
        m["att_in"] = np.ascontiguousarray(att_all[:, :, sl])
        m["gen_in"] = np.ascontiguousarray(
            _regroup(G8[:, :, sl]).astype(np_f8))
        m["genb_in"] = np.ascontiguousarray(_regroup(genb_all[:, :, sl]))
        in_maps.append(m)

    nc = _get_nc()
    res = run_bass_kernel_spmd(nc, in_maps, core_ids=list(range(NCORES)))
    global LAST_RESULTS, LAST_IN_MAPS
    LAST_RESULTS = res
    LAST_IN_MAPS = in_maps

    full = np.concatenate(
        [np.asarray(res.results[c]["out"], np.float32) for c in range(NCORES)],
        axis=1)[:, :T]                                # [32, T], row = p*8+b
    return np.ascontiguousarray(
        full.reshape(NP, B, T).transpose(1, 0, 2).reshape(B, NP * T))


# ---------------------------------------------------------------------------
# Timing harness (test-only): device-resident inputs, repeated execution.
# ---------------------------------------------------------------------------
def benchmark_last(in_maps, iters=8, nc=None):
    import time

    import jax
    from concourse import bass2jax as b2j
    from concourse import mybir as _mybir

    if nc is None:
        nc = _get_nc()
    b2j.install_neuronx_cc_hook()

    partition_name = (nc.partition_id_tensor.name
                      if nc.partition_id_tensor else None)
    in_names, out_names, out_avals, zero_outs = [], [], [], []
    for alloc in nc.m.functions[0].allocations:
        if not isinstance(alloc, _mybir.MemoryLocationSet):
            continue
        name = alloc.memorylocations[0].name
        if alloc.kind == "ExternalInput":
            if name != partition_name:
                in_names.append(name)
        elif alloc.kind == "ExternalOutput":
            shape = tuple(alloc.tensor_shape)
            dtype = _mybir.dt.np(alloc.dtype)
            out_names.append(name)
            out_avals.append(jax.core.ShapedArray(shape, dtype))
            zero_outs.append(np.zeros(shape, dtype))
    n_params = len(in_names)
    n_outs = len(out_avals)
    in_names_all = in_names + out_names
    if partition_name is not None:
        in_names_all.append(partition_name)

    def _body(*args):
        operands = list(args)
        if partition_name is not None:
            operands.append(b2j.partition_id_tensor())
        return tuple(b2j._bass_exec_p.bind(
            *operands,
            out_avals=tuple(out_avals),
            in_names=tuple(in_names_all),
            out_names=tuple(out_names),
            lowering_input_output_aliases=(),
            sim_require_finite=True,
            sim_require_nnan=True,
            nc=nc,
        ))

    donate = tuple(range(n_params, n_params + n_outs))
    devices = jax.devices()[:NCORES]
    mesh = b2j.Mesh(np.asarray(devices), ("core",))
    sharded = jax.jit(
        b2j.shard_map(_body, mesh=mesh,
                      in_specs=(b2j.PartitionSpec("core"),) * (n_params + n_outs),
                      out_specs=(b2j.PartitionSpec("core"),) * n_outs,
                      check_rep=False),
        donate_argnums=donate, keep_unused=True)

    concat_in = [
        np.concatenate([np.asarray(in_maps[c][nm]) for c in range(NCORES)],
                       axis=0)
        for nm in in_names
    ]
    sharding = jax.sharding.NamedSharding(mesh, b2j.PartitionSpec("core"))
    dev_in = [jax.device_put(a, sharding) for a in concat_in]

    def _zeros():
        return [jax.device_put(
            np.zeros((NCORES * z.shape[0], *z.shape[1:]), z.dtype), sharding)
            for z in zero_outs]

    # warmup (compile + load)
    outs = sharded(*dev_in, *_zeros())
    jax.block_until_ready(outs)
    times = []
    for _ in range(iters):
        zs = _zeros()
        jax.block_until_ready(zs)
        t0 = time.perf_counter()
        outs = sharded(*dev_in, *zs)
        jax.block_until_ready(outs)
        times.append(time.perf_counter() - t0)
    return min(times), times


# revision 9
# speedup vs baseline: 1.1573x; 1.1573x over previous
"""Trainium2 Bass kernel for nn_DynamicSelectiveHyperNet.

Strategy
--------
Shard the target-parameter axis T across the 8 NeuronCores (no collectives;
the gated head-sum is computed locally per T-slice). Each core runs all 8
heads for its slice. Streams are fp8-compressed (att: 64 feat rows + rank-4
embeds/bias fold, x128; gen: 4 column groups folded into K=128).

v3 structure (PE-column + LDWEIGHTS + sync-latency aware):
  * att matmuls in fp8 DoubleRow perf mode; the two k-subtiles carry two
    adjacent 512-wide column groups via a block stationary, so 2 DR matmuls
    per (head, supertile) replace 4 plain fp8 matmuls.
  * gen_b2 bias: host-regrouped [H, 4, ts/4] stream + one K=4 matmul
    (block stationary from the gate row) instead of 4 K=1 matmuls.
  * head-sum on PE via identity-matmul PSUM accumulation, delayed by one
    full supertile so PE never stalls on the ACT(sigmoid)->DVE(product)
    chain; the 8 idsums run back-to-back (one shared stationary).
  * DMAs are batched: 1 bf16 const blob, 1 f32 blob, 1 ohdr, 4 att chunk
    DMAs (all heads per chunk), 1 gen, 1 genb, 2 out = ~11 DMAs/iteration.
  * preamble flattened: one matmul for all heads' hmid; lgen2 built via
    DVE transposes/copies (no PSUM, no per-head PE round trips).

The preamble runs inside the repeat loop used for timing, so amortized
per-iteration numbers include it.
"""

import sys

sys.path.insert(0, "/opt/trn_rl_repo")

import json

import numpy as np

import concourse.bass as bass
import concourse.bass2jax as _bass2jax
import concourse.bass_utils as _bass_utils
import concourse.tile as tile
from concourse import mybir
from concourse.bass_utils import run_bass_kernel_spmd

AF = mybir.ActivationFunctionType
ALU = mybir.AluOpType
F32 = mybir.dt.float32
BF16 = mybir.dt.bfloat16
F8 = mybir.dt.float8e4
AX = mybir.AxisListType
DR = mybir.MatmulPerfMode.DoubleRow

B = 8
H = 8
NP = 4          # target param groups
FEAT = 64
EMB = 32
HIN = 96        # FEAT + EMB
GH = 32         # generator hidden
T = 101770
NCORES = 8
TS = 12800      # per-core T shard (8*TS = 102400 >= T, zero padded)
SUP = 2048      # supertile columns (4 col-groups x 512)
NSUB = 512
KFE = 896       # 784 padded to 7*128
PB = NP * B     # 32
KA = FEAT + NP  # 68: att stream rows (feats part + rank-4 embeds/bias fold)
N_SUP = 6       # full supertiles per core; plus one 512-wide tail
CHSUP = 2       # att DMA chunk size in supertiles

SC_W = 128.0    # host scale on att stream values
SC_F = 16.0     # device scale on feats in the att stationary
SC_OH = 16.0    # onehot value (matches SC_F so F rows align with A1 rows)
STAGGERED = True
ATT_DR = False  # att matmuls in fp8 DoubleRow mode (slow on this hw?)

# const blob column offsets (bf16 blob [128, CB_TOT])
CB_FE1 = 0
CB_XT = CB_FE1 + 896
CB_FW2 = CB_XT + 56
CB_GWT = CB_FW2 + 64
CB_SEL4 = CB_GWT + 8
CB_G1 = CB_SEL4 + 32
CB_ID = CB_G1 + 256
CB_EMB = CB_ID + 128
CB_TOT = CB_EMB + 32

# ---------------------------------------------------------------------------
# Workaround: this container's walrus build rejects more than one sync-wait
# command per instruction, while Tile freely attaches several. Split the
# extra waits onto same-engine NoOps inserted just before the instruction.
# ---------------------------------------------------------------------------
_orig_compile_bir_kernel = _bass_utils.compile_bir_kernel


def _split_multi_waits(bir):
    for fn in bir.get("functions", []):
        for bb in fn.get("blocks", []):
            out = []
            for ins in bb.get("instructions", []):
                si = ins.get("sync_info")
                waits = (si or {}).get("on_wait") or []
                if len(waits) > 1:
                    for k, w in enumerate(waits[:-1]):
                        out.append({
                            "debug": ins.get("debug", 0),
                            "engine": ins["engine"],
                            "ins": [],
                            "name": f"{ins['name']}-wsplit{k}",
                            "opcode": "NoOp",
                            "outs": [],
                            "sync_info": {"on_update": [], "on_wait": [w]},
                        })
                    si["on_wait"] = [waits[-1]]
                out.append(ins)
            bb["instructions"] = out
    return bir


def _patched_compile_bir_kernel(bir_json, tmpdir, neff_name="file.neff"):
    bir = _split_multi_waits(json.loads(bir_json))
    return _orig_compile_bir_kernel(json.dumps(bir).encode(), tmpdir,
                                    neff_name=neff_name)


def _install_patch():
    _bass_utils.compile_bir_kernel = _patched_compile_bir_kernel
    _bass2jax.compile_bir_kernel = _patched_compile_bir_kernel


_install_patch()


# ---------------------------------------------------------------------------
# Device program
# ---------------------------------------------------------------------------
def _build_bass(ts=TS, repeats=1):
    nc = bass.Bass()

    att_in = nc.dram_tensor("att_in", [H, KA, ts], F8, kind="ExternalInput")
    gen_in = nc.dram_tensor("gen_in", [H, 4 * GH, ts // 4], F8,
                            kind="ExternalInput")
    genb_in = nc.dram_tensor("genb_in", [H, 4, ts // 4], BF16,
                             kind="ExternalInput")
    cblob = nc.dram_tensor("cblob", [128, CB_TOT], BF16, kind="ExternalInput")
    fblob = nc.dram_tensor("fblob", [128, 2], F32, kind="ExternalInput")
    ohdr = nc.dram_tensor("ohdr", [NP, 512], F8, kind="ExternalInput")
    out = nc.dram_tensor("out", [PB, ts], BF16, kind="ExternalOutput")

    assert ts == N_SUP * SUP + NSUB

    with tile.TileContext(nc) as tc:
        with (
            tc.tile_pool(name="const", bufs=1) as cp,
            tc.tile_pool(name="stream", bufs=1) as sp,
            tc.tile_pool(name="attstream", bufs=2) as ap,
            tc.tile_pool(name="psumA", bufs=3, space="PSUM") as ppA,
            tc.tile_pool(name="psumG", bufs=2, space="PSUM") as ppG,
            tc.tile_pool(name="prepsum", bufs=1, space="PSUM") as prep,
            tc.tile_pool(name="impp", bufs=3) as impp,
            tc.tile_pool(name="tmpp", bufs=16) as tmpp,
            tc.tile_pool(name="accp", bufs=1) as accp,
        ):
            def body():
                _emit_iter(nc, tc, cp, sp, ap, ppA, ppG, prep, impp, tmpp,
                           accp, att_in, gen_in, genb_in, cblob, fblob,
                           ohdr, out)

            if repeats > 1:
                with tc.For_i(0, repeats,
                              staggered_reset=STAGGERED,
                              hint_engines=(mybir.EngineType.PE,
                                            mybir.EngineType.SP,
                                            mybir.EngineType.DVE,
                                            mybir.EngineType.Activation)):
                    body()
            else:
                body()
    return nc


def _emit_iter(nc, tc, cp, sp, ap, ppA, ppG, prep, impp, tmpp, accp,
               att_in, gen_in, genb_in, cblob, fblob, ohdr, out):
    ts = out.shape[1]
    inv_scale = 1.0 / (SC_W * SC_F)
    gsc = 1.0 / SC_W

    # ---- stream DMAs issued first (no preamble dependency) ----------------
    gen8 = sp.tile([4 * GH, H, ts // 4], F8, name="gen8", tag="gen8")
    nc.scalar.dma_start(gen8[:], gen_in.rearrange("h p c -> p h c"))
    genb8 = sp.tile([4, H, ts // 4], BF16, name="genb8", tag="genb8")
    nc.scalar.dma_start(genb8[:], genb_in.rearrange("h p c -> p h c"))

    chunks = [(s0, min(CHSUP, N_SUP + 1 - s0))
              for s0 in range(0, N_SUP + 1, CHSUP)]
    att_tiles = {}

    def issue_att_chunk(ci):
        s0, cnt = chunks[ci]
        cstart = s0 * SUP
        csize = min(CHSUP * SUP, ts - cstart)
        t = ap.tile([KA, H, CHSUP * SUP], F8, name="att8", tag="att8")
        nc.sync.dma_start(t[:, :, :csize],
                          att_in[:, :, cstart:cstart + csize]
                          .rearrange("h p c -> p h c"))
        att_tiles[ci] = t

    issue_att_chunk(0)

    # ---- constant loads (3 DMAs) ------------------------------------------
    blob = cp.tile([128, CB_TOT], BF16, name="blob", tag="blob")
    nc.sync.dma_start(blob[:], cblob[:])
    fb = cp.tile([128, 2], F32, name="fb", tag="fb")
    nc.sync.dma_start(fb[:], fblob[:])

    fe1_t = blob[:, CB_FE1:CB_XT].rearrange("p (o m) -> p o m", o=7)
    xt_t = blob[:, CB_XT:CB_FW2].rearrange("p (o m) -> p o m", o=7)
    fw2_t = blob[:, CB_FW2:CB_GWT]
    gwt_t = blob[:FEAT + 1, CB_GWT:CB_SEL4]
    sel4_t = blob[:B, CB_SEL4:CB_G1]
    g1_t = blob[:HIN + 1, CB_G1:CB_ID]
    id_t = blob[:, CB_ID:CB_EMB]
    emb_t = blob[:EMB, CB_EMB:CB_TOT]

    issue_att_chunk(1)

    hinT = cp.tile([HIN + 1, PB], BF16)     # [97, 32] stationary (gen_W1)
    lgen2 = cp.tile([128, H * 128], BF16, name="lgen2", tag="lgen2")
    S4all = cp.tile([4, H * 128], BF16, name="S4all", tag="S4all")
    nc.vector.memset(S4all[:], 0.0)
    # att DoubleRow stationary: two [68, 2, 128] tiles side by side; subtile
    # i of tile t holds hinF at output block 32*(2t+i)
    attStat = cp.tile([KA, 512], F8, name="attStat", tag="attStat")
    nc.vector.memset(attStat[:], 0.0)

    # ---- feature extractor ------------------------------------------------
    psf = prep.tile([128, 256], F32, tag="pre1")
    for o in range(7):
        nc.tensor.matmul(psf[:, :B], fe1_t[:, o, :], xt_t[:, o, :],
                         start=(o == 0), stop=(o == 6))
    relu1 = cp.tile([128, B], BF16)
    nc.scalar.activation(relu1[:], psf[:, :B], AF.Relu, bias=fb[:, 0:1])

    psf2 = prep.tile([128, 256], F32, tag="pre1")
    nc.tensor.matmul(psf2[:FEAT, :B], fw2_t, relu1[:], start=True, stop=True)
    featsT = cp.tile([FEAT + 1, B], BF16)   # [65, 8], row 64 = ones
    nc.scalar.activation(featsT[:FEAT, :], psf2[:FEAT, :B], AF.Identity,
                         bias=fb[:FEAT, 1:2])
    nc.vector.memset(featsT[FEAT:FEAT + 1, :], 1.0)

    # ---- head gate (softmax over heads, normalization folded) -------------
    psgl = prep.tile([128, 256], F32, tag="pre1")
    nc.tensor.matmul(psgl[:B, :B], featsT[:], gwt_t, start=True, stop=True)
    gateb = cp.tile([32, 32], F32)          # gate[b, h] in [0:8, 0:8]
    nc.vector.memset(gateb[:], 0.0)
    nc.scalar.activation(gateb[:B, :B], psgl[:B, :B], AF.Exp)
    sums = cp.tile([B, 1], F32)
    nc.vector.tensor_reduce(sums[:], gateb[:B, :B], AX.X, ALU.add)
    recip = cp.tile([B, 1], F32)
    nc.vector.reciprocal(recip[:], sums[:])
    nc.vector.tensor_scalar_mul(gateb[:B, :B], gateb[:B, :B], recip[:])
    gatebT = cp.tile([32, 32], F32)         # gate[h, b] in [0:8, 0:8]
    nc.vector.transpose(gatebT[:], gateb[:])
    gatebT_bf = cp.tile([32, 32], BF16)
    nc.vector.tensor_copy(gatebT_bf[:], gatebT[:])
    # gate column per (pb, h): gcols[pb, h] = gate[h, pb % 8]
    psgc = prep.tile([128, 256], F32, tag="pre1")
    nc.tensor.matmul(psgc[:PB, :B], sel4_t, gatebT_bf[:B, :B],
                     start=True, stop=True)
    gcols = cp.tile([PB, B], F32)
    nc.vector.tensor_copy(gcols[:], psgc[:PB, :B])

    # ---- S4all: S4[g, 128h+32g+pb] = gate[h, pb]*SC_W ---------------------
    gcwt = cp.tile([B, PB], BF16, name="gcwt", tag="gcwt")
    for p in range(NP):
        nc.scalar.mul(gcwt[:, p * B:(p + 1) * B], gateb[:B, :B],
                      float(SC_W))
    for g in range(4):
        dst = S4all[g:g + 1, :].rearrange("p (h c) -> p h c", c=128)
        dst = dst[:, :, 32 * g:32 * g + PB]
        nc.sync.dma_start(dst, gcwt[:])

    # ---- hinT (stationary for the gen_W1 matmuls) -------------------------
    for p in range(NP):
        nc.vector.tensor_copy(hinT[:FEAT, p * B:(p + 1) * B],
                              featsT[:FEAT, :])
    nc.vector.tensor_copy(hinT[FEAT:HIN, :], emb_t)
    nc.vector.memset(hinT[HIN:HIN + 1, :], 1.0)

    # ---- attStat: hinF feats block (x SC_F) replicated to 4 positions -----
    hinf_s = cp.tile([FEAT, PB], F8, name="hinf_s", tag="hinf_s")
    for p in range(NP):
        nc.scalar.mul(hinf_s[:, p * B:(p + 1) * B], featsT[:FEAT, :],
                      float(SC_F))
    for t2 in range(2):
        for i in range(2):
            g = 2 * t2 + i
            o = 256 * t2 + 128 * i + 32 * g
            nc.vector.tensor_copy(attStat[:FEAT, o:o + 32], hinf_s[:])
    nc.sync.dma_start(attStat[FEAT:KA, :], ohdr[:])
    if not ATT_DR:
        hinF = cp.tile([KA, PB], F8, name="hinF", tag="hinF")
        nc.vector.tensor_copy(hinF[:FEAT, :], hinf_s[:])
        nc.sync.dma_start(hinF[FEAT:KA, :], ohdr[:, 0:PB])

    # ---- all heads' hmid in one shot; lgen2 via DVE transpose/copies ------
    psh = prep.tile([128, 256], F32, tag="pre1")
    nc.tensor.matmul(psh[:PB, :], hinT[:], g1_t, start=True, stop=True)
    hmid_all = cp.tile([PB, H * GH], F32, name="hmid_all", tag="hmid_all")
    nc.scalar.activation(hmid_all[:], psh[:PB, :], AF.Relu)
    for h in range(H):
        nc.vector.tensor_scalar_mul(hmid_all[:, h * GH:(h + 1) * GH],
                                    hmid_all[:, h * GH:(h + 1) * GH],
                                    gcols[:, h:h + 1])
    hmidT_all = cp.tile([PB, H * GH], F32, name="hmidT_all", tag="hmidT_all")
    for h in range(H):
        nc.vector.transpose(hmidT_all[:, h * GH:(h + 1) * GH],
                            hmid_all[:, h * GH:(h + 1) * GH])
    nc.vector.memset(lgen2[:], 0.0)
    for h in range(H):
        for g in range(4):
            nc.vector.tensor_copy(
                lgen2[32 * g:32 * (g + 1),
                      h * 128 + 32 * g:h * 128 + 32 * (g + 1)],
                hmidT_all[:, h * GH:(h + 1) * GH])

    # ---- main loop: s-outer / h-inner; idsum delayed one supertile --------
    accout = accp.tile([128, ts // 4], BF16, name="accout", tag="accb")

    def emit_idsum(prev_s, prev_tmps):
        pncols = SUP if prev_s < N_SUP else NSUB
        pns = pncols // 4
        psC = ppG.tile([128, NSUB], F32, tag="psC")
        for h in range(H):
            nc.tensor.matmul(psC[:, :pns], id_t, prev_tmps[h][:, :pns],
                             start=(h == 0), stop=(h == H - 1))
        a0 = prev_s * NSUB
        nc.scalar.copy(accout[:, a0:a0 + pns], psC[:, :pns])

    prev = None
    for ci, (s0, cnt) in enumerate(chunks):
        if ci + 2 < len(chunks):
            issue_att_chunk(ci + 2)
        for s in range(s0, s0 + cnt):
            ncols = SUP if s < N_SUP else NSUB
            ns = ncols // 4
            rel = (s - s0) * SUP
            q0 = s * NSUB
            att_t = att_tiles[ci]
            tmps = []
            for h in range(H):
                psA = ppA.tile([128, NSUB], F32, tag="psA")
                psG = ppG.tile([128, NSUB], F32, tag="psG")
                if ATT_DR:
                    for t2 in range(2):
                        o = rel + 2 * t2 * ns
                        nc.tensor.matmul(
                            psA[:, :ns],
                            attStat[:, 256 * t2:256 * (t2 + 1)]
                            .rearrange("p (i c) -> p i c", i=2),
                            att_t[:, h, o:o + 2 * ns]
                            .rearrange("p (i n) -> p i n", i=2),
                            start=(t2 == 0), stop=(t2 == 1),
                            perf_mode=DR)
                else:
                    for g in range(4):
                        nc.tensor.matmul(
                            psA[32 * g:32 * (g + 1), :ns], hinF[:],
                            att_t[:, h, rel + g * ns:rel + (g + 1) * ns],
                            start=True, stop=True,
                            tile_position=(0, 32 * g))
                nc.tensor.matmul(psG[:, :ns], lgen2[:, h * 128:(h + 1) * 128],
                                 gen8[:, h, q0:q0 + ns],
                                 start=True, stop=False)
                nc.tensor.matmul(psG[:, :ns], S4all[:, h * 128:(h + 1) * 128],
                                 genb8[:, h, q0:q0 + ns],
                                 start=False, stop=True)
                imp = impp.tile([128, NSUB], F32, tag="imp")
                nc.scalar.activation(imp[:, :ns], psA[:, :ns], AF.Sigmoid,
                                     scale=float(inv_scale))
                tmp = tmpp.tile([128, NSUB], BF16, tag="tmp")
                nc.vector.scalar_tensor_tensor(tmp[:, :ns], psG[:, :ns],
                                               float(gsc), imp[:, :ns],
                                               ALU.mult, ALU.mult)
                tmps.append(tmp)
            if prev is not None:
                emit_idsum(*prev)
            prev = (s, tmps)
    emit_idsum(*prev)

    # ---- output writeback (one DMA per supertile) -------------------------
    for s in range(N_SUP + 1):
        ncols = SUP if s < N_SUP else NSUB
        ns = ncols // 4
        nc.scalar.dma_start(
            out[:, s * SUP:s * SUP + ncols].rearrange("p (g c) -> g p c", g=4),
            accout[:, s * NSUB:s * NSUB + ns])
    return nc


_NC_CACHE = None


def _get_nc():
    global _NC_CACHE
    if _NC_CACHE is None:
        _NC_CACHE = _build_bass()
    return _NC_CACHE


# ---------------------------------------------------------------------------
# Host wrapper
# ---------------------------------------------------------------------------
LAST_RESULTS = None
LAST_IN_MAPS = None


def kernel(x, fe_W1, fe_b1, fe_W2, fe_b2, embeds,
           gen_W1, gen_b1, gen_W2, gen_b2, att_W, att_b,
           gate_W, gate_b):
    f32 = np.float32
    np_f8 = mybir.dt.np(F8)
    np_bf = mybir.dt.np(BF16)
    x = np.asarray(x, f32)
    fe_W1 = np.asarray(fe_W1, f32)
    fe_b1 = np.asarray(fe_b1, f32)
    fe_W2 = np.asarray(fe_W2, f32)
    fe_b2 = np.asarray(fe_b2, f32)
    embeds = np.asarray(embeds, f32)
    gen_W1 = np.asarray(gen_W1, f32)
    gen_b1 = np.asarray(gen_b1, f32)
    gen_W2 = np.asarray(gen_W2, f32)
    gen_b2 = np.asarray(gen_b2, f32)
    att_W = np.asarray(att_W, f32)
    att_b = np.asarray(att_b, f32)
    gate_W = np.asarray(gate_W, f32)
    gate_b = np.asarray(gate_b, f32)

    # --- att stream: [H, 68, T_pad] (feats rows + rank-4 embeds/bias fold) --
    tpad = NCORES * TS
    att_all = np.zeros((H, KA, tpad), np_f8)
    att_all[:, :FEAT, :T] = (att_W[:, :, :FEAT].transpose(0, 2, 1)
                             * SC_W).astype(np_f8)
    F = np.einsum('pj,htj->hpt', embeds, att_W[:, :, FEAT:]) \
        + att_b[:, None, :]
    att_all[:, FEAT:, :T] = (F * SC_W).astype(np_f8)
    # --- gen stream ---------------------------------------------------------
    G8 = np.zeros((H, GH, tpad), np.float32)
    G8[:, :, :T] = gen_W2.transpose(0, 2, 1) * SC_W
    genb_all = np.zeros((H, 1, tpad), np_bf)
    genb_all[:, 0, :T] = gen_b2.astype(np_bf)

    # --- const blobs ---
    xt = np.zeros((KFE, B), f32)
    xt[:784] = x.T
    fe1t = np.zeros((KFE, 128), f32)
    fe1t[:784] = fe_W1.T
    gwt = np.concatenate([gate_W.T, gate_b[None, :]], axis=0)
    emb = np.repeat(embeds.T[:, :, None], B, axis=2).reshape(EMB, PB)
    sel4 = np.tile(np.eye(B, dtype=f32), NP)
    g1in = np.concatenate([gen_W1.transpose(0, 2, 1), gen_b1[:, None, :]],
                          axis=1)                      # [H, 97, 32]
    g1in = g1in.transpose(1, 0, 2).reshape(HIN + 1, H * GH)

    def pad128(a):
        o = np.zeros((128, a.shape[1]), f32)
        o[:a.shape[0]] = a
        return o

    cb = np.zeros((128, CB_TOT), f32)
    cb[:, CB_FE1:CB_XT] = fe1t.reshape(7, 128, 128).transpose(1, 0, 2) \
        .reshape(128, 896)
    cb[:, CB_XT:CB_FW2] = xt.reshape(7, 128, B).transpose(1, 0, 2) \
        .reshape(128, 56)
    cb[:, CB_FW2:CB_GWT] = fe_W2.T
    cb[:, CB_GWT:CB_SEL4] = pad128(gwt)
    cb[:, CB_SEL4:CB_G1] = pad128(sel4)
    cb[:, CB_G1:CB_ID] = pad128(g1in)
    cb[:, CB_ID:CB_EMB] = np.eye(128, dtype=f32)
    cb[:, CB_EMB:CB_TOT] = pad128(emb)

    fblob_a = np.zeros((128, 2), f32)
    fblob_a[:, 0] = fe_b1
    fblob_a[:FEAT, 1] = fe_b2

    # onehot rows for the DoubleRow att stationary: [4, 512]; subtile i of
    # tile t carries the onehot block at columns 256t+128i+32(2t+i)
    oh = np.repeat(np.eye(NP, dtype=f32), B, axis=1) * SC_OH   # [4, 32]
    ohdr_a = np.zeros((NP, 512), f32)
    for t2 in range(2):
        for i in range(2):
            g = 2 * t2 + i
            o = 256 * t2 + 128 * i + 32 * g
            ohdr_a[:, o:o + PB] = oh

    shared = {
        "cblob": np.ascontiguousarray(cb.astype(np_bf)),
        "fblob": fblob_a,
        "ohdr": np.ascontiguousarray(ohdr_a.astype(np_f8)),
    }

    def _regroup(a):
        # [H, k, TS] -> [H, 4k, TS//4]: column group g of each supertile
        # becomes contraction rows [g*k, (g+1)*k)
        k = a.shape[1]
        o = np.zeros((H, 4 * k, TS // 4), a.dtype)
        for s in range(7):
            c0 = s * SUP
            ns = (SUP if s < 6 else NSUB) // 4
            for g in range(4):
                o[:, g * k:(g + 1) * k, s * NSUB:s * NSUB + ns] = \
                    a[:, :, c0 + g * ns:c0 + (g + 1) * ns]
        return o

    in_maps = []
    for c in range(NCORES):
        sl = slice(c * TS, (c + 1) * TS)
        m = dict(shared)
        m["att_in"] = np.ascontiguousarray(att_all[:, :, sl])
        m["gen_in"] = np.ascontiguousarray(
            _regroup(G8[:, :, sl]).astype(np_f8))
        m["genb_in"] = np.ascontiguousarray(_regroup(genb_all[:, :, sl]))
        in_maps.append(m)

    nc = _get_nc()
    res = run_bass_kernel_spmd(nc, in_maps, core_ids=list(range(NCORES)))
    global LAST_RESULTS, LAST_IN_MAPS
    LAST_RESULTS = res
    LAST_IN_MAPS = in_maps

    full = np.concatenate(
        [np.asarray(res.results[c]["out"], np.float32) for c in range(NCORES)],
        axis=1)[:, :T]                                # [32, T], row = p*8+b
    return np.ascontiguousarray(
        full.reshape(NP, B, T).transpose(1, 0, 2).reshape(B, NP * T))


# ---------------------------------------------------------------------------
# Timing harness (test-only): device-resident inputs, repeated execution.
# ---------------------------------------------------------------------------
def benchmark_last(in_maps, iters=8, nc=None):
    import time

    import jax
    from concourse import bass2jax as b2j
    from concourse import mybir as _mybir

    if nc is None:
        nc = _get_nc()
    b2j.install_neuronx_cc_hook()

    partition_name = (nc.partition_id_tensor.name
                      if nc.partition_id_tensor else None)
    in_names, out_names, out_avals, zero_outs = [], [], [], []
    for alloc in nc.m.functions[0].allocations:
        if not isinstance(alloc, _mybir.MemoryLocationSet):
            continue
        name = alloc.memorylocations[0].name
        if alloc.kind == "ExternalInput":
            if name != partition_name:
                in_names.append(name)
        elif alloc.kind == "ExternalOutput":
            shape = tuple(alloc.tensor_shape)
            dtype = _mybir.dt.np(alloc.dtype)
            out_names.append(name)
            out_avals.append(jax.core.ShapedArray(shape, dtype))
            zero_outs.append(np.zeros(shape, dtype))
    n_params = len(in_names)
    n_outs = len(out_avals)
    in_names_all = in_names + out_names
    if partition_name is not None:
        in_names_all.append(partition_name)

    def _body(*args):
        operands = list(args)
        if partition_name is not None:
            operands.append(b2j.partition_id_tensor())
        return tuple(b2j._bass_exec_p.bind(
            *operands,
            out_avals=tuple(out_avals),
            in_names=tuple(in_names_all),
            out_names=tuple(out_names),
            lowering_input_output_aliases=(),
            sim_require_finite=True,
            sim_require_nnan=True,
            nc=nc,
        ))

    donate = tuple(range(n_params, n_params + n_outs))
    devices = jax.devices()[:NCORES]
    mesh = b2j.Mesh(np.asarray(devices), ("core",))
    sharded = jax.jit(
        b2j.shard_map(_body, mesh=mesh,
                      in_specs=(b2j.PartitionSpec("core"),) * (n_params + n_outs),
                      out_specs=(b2j.PartitionSpec("core"),) * n_outs,
                      check_rep=False),
        donate_argnums=donate, keep_unused=True)

    concat_in = [
        np.concatenate([np.asarray(in_maps[c][nm]) for c in range(NCORES)],
                       axis=0)
        for nm in in_names
    ]
    sharding = jax.sharding.NamedSharding(mesh, b2j.PartitionSpec("core"))
    dev_in = [jax.device_put(a, sharding) for a in concat_in]

    def _zeros():
        return [jax.device_put(
            np.zeros((NCORES * z.shape[0], *z.shape[1:]), z.dtype), sharding)
            for z in zero_outs]

    # warmup (compile + load)
    outs = sharded(*dev_in, *_zeros())
    jax.block_until_ready(outs)
    times = []
    for _ in range(iters):
        zs = _zeros()
        jax.block_until_ready(zs)
        t0 = time.perf_counter()
        outs = sharded(*dev_in, *zs)
        jax.block_until_ready(outs)
        times.append(time.perf_counter() - t0)
    return min(times), times


# revision 13
# speedup vs baseline: 1.6895x; 1.4599x over previous
"""Trainium2 Bass kernel for nn_DynamicSelectiveHyperNet.

Strategy
--------
Shard the target-parameter axis T across the 8 NeuronCores (no collectives;
the gated head-sum is computed locally per T-slice). Each core runs all 8
heads for its slice. Streams are fp8-compressed (att: 64 feat rows + rank-4
embeds/bias fold, x128; gen: 4 column groups folded into K=128).

v3 structure (PE-column + LDWEIGHTS + sync-latency aware):
  * att matmuls in fp8 DoubleRow perf mode; the two k-subtiles carry two
    adjacent 512-wide column groups via a block stationary, so 2 DR matmuls
    per (head, supertile) replace 4 plain fp8 matmuls.
  * gen_b2 bias: host-regrouped [H, 4, ts/4] stream + one K=4 matmul
    (block stationary from the gate row) instead of 4 K=1 matmuls.
  * head-sum on PE via identity-matmul PSUM accumulation, delayed by one
    full supertile so PE never stalls on the ACT(sigmoid)->DVE(product)
    chain; the 8 idsums run back-to-back (one shared stationary).
  * DMAs are batched: 1 bf16 const blob, 1 f32 blob, 1 ohdr, 4 att chunk
    DMAs (all heads per chunk), 1 gen, 1 genb, 2 out = ~11 DMAs/iteration.
  * preamble flattened: one matmul for all heads' hmid; lgen2 built via
    DVE transposes/copies (no PSUM, no per-head PE round trips).

The preamble runs inside the repeat loop used for timing, so amortized
per-iteration numbers include it.
"""

import sys

sys.path.insert(0, "/opt/trn_rl_repo")

import json

import numpy as np

import concourse.bass as bass
import concourse.bass2jax as _bass2jax
import concourse.bass_utils as _bass_utils
import concourse.tile as tile
from concourse import mybir
from concourse.bass_utils import run_bass_kernel_spmd

AF = mybir.ActivationFunctionType
ALU = mybir.AluOpType
F32 = mybir.dt.float32
BF16 = mybir.dt.bfloat16
F8 = mybir.dt.float8e4
AX = mybir.AxisListType
DR = mybir.MatmulPerfMode.DoubleRow

B = 8
H = 8
NP = 4          # target param groups
FEAT = 64
EMB = 32
HIN = 96        # FEAT + EMB
GH = 32         # generator hidden
T = 101770
NCORES = 8
TS = 12800      # per-core T shard (8*TS = 102400 >= T, zero padded)
SUP = 2048      # supertile columns (4 col-groups x 512)
NSUB = 512
KFE = 896       # 784 padded to 7*128
PB = NP * B     # 32
KA = FEAT + NP  # 68: att stream rows (feats part + rank-4 embeds/bias fold)
N_SUP = 6       # full supertiles per core; plus one 512-wide tail
CHSUP = 2       # att DMA chunk size in supertiles

SC_W = 128.0    # host scale on att stream values
SC_F = 16.0     # device scale on feats in the att stationary
SC_OH = 16.0    # onehot value (matches SC_F so F rows align with A1 rows)
STAGGERED = True
ATT_DR = False  # att matmuls in fp8 DoubleRow mode (slow on this hw?)
import os as _os
ABLATE = _os.environ.get("KABLATE", "full")  # full | pe (matmuls+DMA only)

# const blob column offsets (bf16 blob [128, CB_TOT])
CB_FE1 = 0
CB_XT = CB_FE1 + 896
CB_FW2 = CB_XT + 56
CB_GWT = CB_FW2 + 64
CB_SEL4 = CB_GWT + 8
CB_G1 = CB_SEL4 + 32
CB_ID = CB_G1 + 256
CB_EMB = CB_ID + 128
CB_TOT = CB_EMB + 32

# ---------------------------------------------------------------------------
# Workaround: this container's walrus build rejects more than one sync-wait
# command per instruction, while Tile freely attaches several. Split the
# extra waits onto same-engine NoOps inserted just before the instruction.
# ---------------------------------------------------------------------------
_orig_compile_bir_kernel = _bass_utils.compile_bir_kernel


def _split_multi_waits(bir):
    for fn in bir.get("functions", []):
        for bb in fn.get("blocks", []):
            out = []
            for ins in bb.get("instructions", []):
                si = ins.get("sync_info")
                waits = (si or {}).get("on_wait") or []
                if len(waits) > 1:
                    for k, w in enumerate(waits[:-1]):
                        out.append({
                            "debug": ins.get("debug", 0),
                            "engine": ins["engine"],
                            "ins": [],
                            "name": f"{ins['name']}-wsplit{k}",
                            "opcode": "NoOp",
                            "outs": [],
                            "sync_info": {"on_update": [], "on_wait": [w]},
                        })
                    si["on_wait"] = [waits[-1]]
                out.append(ins)
            bb["instructions"] = out
    return bir


def _patched_compile_bir_kernel(bir_json, tmpdir, neff_name="file.neff"):
    bir = _split_multi_waits(json.loads(bir_json))
    return _orig_compile_bir_kernel(json.dumps(bir).encode(), tmpdir,
                                    neff_name=neff_name)


def _install_patch():
    _bass_utils.compile_bir_kernel = _patched_compile_bir_kernel
    _bass2jax.compile_bir_kernel = _patched_compile_bir_kernel


_install_patch()


# ---------------------------------------------------------------------------
# Device program
# ---------------------------------------------------------------------------
def _build_bass(ts=TS, repeats=1):
    nc = bass.Bass()

    att_in = nc.dram_tensor("att_in", [H, KA, ts], F8, kind="ExternalInput")
    gen_in = nc.dram_tensor("gen_in", [H, 4 * GH, ts // 4], F8,
                            kind="ExternalInput")
    genb_in = nc.dram_tensor("genb_in", [H, 4, ts // 4], BF16,
                             kind="ExternalInput")
    cblob = nc.dram_tensor("cblob", [128, CB_TOT], BF16, kind="ExternalInput")
    fblob = nc.dram_tensor("fblob", [128, 2], F32, kind="ExternalInput")
    ohdr = nc.dram_tensor("ohdr", [NP, 512], F8, kind="ExternalInput")
    out = nc.dram_tensor("out", [PB, ts], BF16, kind="ExternalOutput")

    assert ts == N_SUP * SUP + NSUB

    with tile.TileContext(nc) as tc:
        with (
            tc.tile_pool(name="const", bufs=1) as cp,
            tc.tile_pool(name="stream", bufs=1) as sp,
            tc.tile_pool(name="attstream", bufs=2) as ap,
            tc.tile_pool(name="psumA", bufs=3, space="PSUM") as ppA,
            tc.tile_pool(name="psumG", bufs=2, space="PSUM") as ppG,
            tc.tile_pool(name="prepsum", bufs=1, space="PSUM") as prep,
            tc.tile_pool(name="impp", bufs=3) as impp,
            tc.tile_pool(name="tmpp", bufs=16) as tmpp,
            tc.tile_pool(name="accp", bufs=1) as accp,
        ):
            def body():
                _emit_iter(nc, tc, cp, sp, ap, ppA, ppG, prep, impp, tmpp,
                           accp, att_in, gen_in, genb_in, cblob, fblob,
                           ohdr, out)

            if repeats > 1:
                with tc.For_i(0, repeats,
                              staggered_reset=STAGGERED,
                              hint_engines=(mybir.EngineType.PE,
                                            mybir.EngineType.SP,
                                            mybir.EngineType.DVE,
                                            mybir.EngineType.Activation)):
                    body()
            else:
                body()
    return nc


def _emit_iter(nc, tc, cp, sp, ap, ppA, ppG, prep, impp, tmpp, accp,
               att_in, gen_in, genb_in, cblob, fblob, ohdr, out):
    ts = out.shape[1]
    inv_scale = 1.0 / (SC_W * SC_F)
    gsc = 1.0 / SC_W

    # ---- stream DMAs issued first (no preamble dependency) ----------------
    gen8 = sp.tile([4 * GH, H, ts // 4], F8, name="gen8", tag="gen8")
    nc.scalar.dma_start(gen8[:], gen_in.rearrange("h p c -> p h c"))
    genb8 = sp.tile([4, H, ts // 4], BF16, name="genb8", tag="genb8")
    nc.scalar.dma_start(genb8[:], genb_in.rearrange("h p c -> p h c"))

    chunks = [(s0, min(CHSUP, N_SUP + 1 - s0))
              for s0 in range(0, N_SUP + 1, CHSUP)]
    att_tiles = {}

    def issue_att_chunk(ci):
        s0, cnt = chunks[ci]
        cstart = s0 * SUP
        csize = min(CHSUP * SUP, ts - cstart)
        t = ap.tile([KA, H, CHSUP * SUP], F8, name="att8", tag="att8")
        nc.sync.dma_start(t[:, :, :csize],
                          att_in[:, :, cstart:cstart + csize]
                          .rearrange("h p c -> p h c"))
        att_tiles[ci] = t

    issue_att_chunk(0)

    # ---- constant loads (3 DMAs) ------------------------------------------
    blob = cp.tile([128, CB_TOT], BF16, name="blob", tag="blob")
    nc.sync.dma_start(blob[:], cblob[:])
    fb = cp.tile([128, 2], F32, name="fb", tag="fb")
    nc.sync.dma_start(fb[:], fblob[:])

    fe1_t = blob[:, CB_FE1:CB_XT].rearrange("p (o m) -> p o m", o=7)
    xt_t = blob[:, CB_XT:CB_FW2].rearrange("p (o m) -> p o m", o=7)
    fw2_t = blob[:, CB_FW2:CB_GWT]
    gwt_t = blob[:FEAT + 1, CB_GWT:CB_SEL4]
    sel4_t = blob[:B, CB_SEL4:CB_G1]
    g1_t = blob[:HIN + 1, CB_G1:CB_ID]
    id_t = blob[:, CB_ID:CB_EMB]
    emb_t = blob[:EMB, CB_EMB:CB_TOT]

    issue_att_chunk(1)

    hinT = cp.tile([HIN + 1, PB], BF16)     # [97, 32] stationary (gen_W1)
    lgen2 = cp.tile([128, H * 128], BF16, name="lgen2", tag="lgen2")
    S4all = cp.tile([4, H * 128], BF16, name="S4all", tag="S4all")
    nc.vector.memset(S4all[:], 0.0)
    # att DoubleRow stationary: two [68, 2, 128] tiles side by side; subtile
    # i of tile t holds hinF at output block 32*(2t+i)
    attStat = cp.tile([KA, 512], F8, name="attStat", tag="attStat")
    nc.vector.memset(attStat[:], 0.0)

    # ---- feature extractor ------------------------------------------------
    psf = prep.tile([128, 256], F32, tag="pre1")
    for o in range(7):
        nc.tensor.matmul(psf[:, :B], fe1_t[:, o, :], xt_t[:, o, :],
                         start=(o == 0), stop=(o == 6))
    relu1 = cp.tile([128, B], BF16)
    nc.scalar.activation(relu1[:], psf[:, :B], AF.Relu, bias=fb[:, 0:1])

    psf2 = prep.tile([128, 256], F32, tag="pre1")
    nc.tensor.matmul(psf2[:FEAT, :B], fw2_t, relu1[:], start=True, stop=True)
    featsT = cp.tile([FEAT + 1, B], BF16)   # [65, 8], row 64 = ones
    nc.scalar.activation(featsT[:FEAT, :], psf2[:FEAT, :B], AF.Identity,
                         bias=fb[:FEAT, 1:2])
    nc.vector.memset(featsT[FEAT:FEAT + 1, :], 1.0)

    # ---- head gate (softmax over heads, normalization folded) -------------
    psgl = prep.tile([128, 256], F32, tag="pre1")
    nc.tensor.matmul(psgl[:B, :B], featsT[:], gwt_t, start=True, stop=True)
    gateb = cp.tile([32, 32], F32)          # gate[b, h] in [0:8, 0:8]
    nc.vector.memset(gateb[:], 0.0)
    nc.scalar.activation(gateb[:B, :B], psgl[:B, :B], AF.Exp)
    sums = cp.tile([B, 1], F32)
    nc.vector.tensor_reduce(sums[:], gateb[:B, :B], AX.X, ALU.add)
    recip = cp.tile([B, 1], F32)
    nc.vector.reciprocal(recip[:], sums[:])
    nc.vector.tensor_scalar_mul(gateb[:B, :B], gateb[:B, :B], recip[:])
    gatebT = cp.tile([32, 32], F32)         # gate[h, b] in [0:8, 0:8]
    nc.vector.transpose(gatebT[:], gateb[:])
    gatebT_bf = cp.tile([32, 32], BF16)
    nc.vector.tensor_copy(gatebT_bf[:], gatebT[:])
    # gate column per (pb, h): gcols[pb, h] = gate[h, pb % 8]
    psgc = prep.tile([128, 256], F32, tag="pre1")
    nc.tensor.matmul(psgc[:PB, :B], sel4_t, gatebT_bf[:B, :B],
                     start=True, stop=True)
    gcols = cp.tile([PB, B], F32)
    nc.vector.tensor_copy(gcols[:], psgc[:PB, :B])

    # ---- S4all: S4[g, 128h+32g+pb] = gate[h, pb]*SC_W ---------------------
    gcwt = cp.tile([B, PB], BF16, name="gcwt", tag="gcwt")
    for p in range(NP):
        nc.scalar.mul(gcwt[:, p * B:(p + 1) * B], gateb[:B, :B],
                      float(SC_W))
    for g in range(4):
        dst = S4all[g:g + 1, :].rearrange("p (h c) -> p h c", c=128)
        dst = dst[:, :, 32 * g:32 * g + PB]
        nc.sync.dma_start(dst, gcwt[:])

    # ---- hinT (stationary for the gen_W1 matmuls) -------------------------
    for p in range(NP):
        nc.vector.tensor_copy(hinT[:FEAT, p * B:(p + 1) * B],
                              featsT[:FEAT, :])
    nc.vector.tensor_copy(hinT[FEAT:HIN, :], emb_t)
    nc.vector.memset(hinT[HIN:HIN + 1, :], 1.0)

    # ---- attStat: hinF feats block (x SC_F) replicated to 4 positions -----
    hinf_s = cp.tile([FEAT, PB], F8, name="hinf_s", tag="hinf_s")
    for p in range(NP):
        nc.scalar.mul(hinf_s[:, p * B:(p + 1) * B], featsT[:FEAT, :],
                      float(SC_F))
    for t2 in range(2):
        for i in range(2):
            g = 2 * t2 + i
            o = 256 * t2 + 128 * i + 32 * g
            nc.vector.tensor_copy(attStat[:FEAT, o:o + 32], hinf_s[:])
    nc.sync.dma_start(attStat[FEAT:KA, :], ohdr[:])
    if not ATT_DR:
        hinF = cp.tile([KA, PB], F8, name="hinF", tag="hinF")
        nc.vector.tensor_copy(hinF[:FEAT, :], hinf_s[:])
        nc.sync.dma_start(hinF[FEAT:KA, :], ohdr[:, 0:PB])

    # ---- all heads' hmid in one shot; lgen2 via DVE transpose/copies ------
    psh = prep.tile([128, 256], F32, tag="pre1")
    nc.tensor.matmul(psh[:PB, :], hinT[:], g1_t, start=True, stop=True)
    hmid_all = cp.tile([PB, H * GH], F32, name="hmid_all", tag="hmid_all")
    nc.scalar.activation(hmid_all[:], psh[:PB, :], AF.Relu)
    for h in range(H):
        nc.vector.tensor_scalar_mul(hmid_all[:, h * GH:(h + 1) * GH],
                                    hmid_all[:, h * GH:(h + 1) * GH],
                                    gcols[:, h:h + 1])
    hmidT_all = cp.tile([PB, H * GH], F32, name="hmidT_all", tag="hmidT_all")
    for h in range(H):
        nc.vector.transpose(hmidT_all[:, h * GH:(h + 1) * GH],
                            hmid_all[:, h * GH:(h + 1) * GH])
    nc.vector.memset(lgen2[:], 0.0)
    for h in range(H):
        for g in range(4):
            nc.vector.tensor_copy(
                lgen2[32 * g:32 * (g + 1),
                      h * 128 + 32 * g:h * 128 + 32 * (g + 1)],
                hmidT_all[:, h * GH:(h + 1) * GH])

    # ---- main loop: s-outer / h-inner; idsum delayed one supertile --------
    accout = accp.tile([128, ts // 4], BF16, name="accout", tag="accb")

    def emit_idsum(prev_s, prev_tmps):
        pncols = SUP if prev_s < N_SUP else NSUB
        pns = pncols // 4
        psC = ppG.tile([128, NSUB], F32, tag="psC")
        for h in range(H):
            nc.tensor.matmul(psC[:, :pns], id_t, prev_tmps[h][:, :pns],
                             start=(h == 0), stop=(h == H - 1))
        a0 = prev_s * NSUB
        nc.scalar.copy(accout[:, a0:a0 + pns], psC[:, :pns])

    prev = None
    for ci, (s0, cnt) in enumerate(chunks):
        if ci + 2 < len(chunks):
            issue_att_chunk(ci + 2)
        for s in range(s0, s0 + cnt):
            ncols = SUP if s < N_SUP else NSUB
            ns = ncols // 4
            rel = (s - s0) * SUP
            q0 = s * NSUB
            att_t = att_tiles[ci]
            tmps = []
            for h in range(H if ABLATE != "dma" else 0):
                psA = ppA.tile([128, NSUB], F32, tag="psA")
                psG = ppG.tile([128, NSUB], F32, tag="psG")
                if ATT_DR:
                    for t2 in range(2):
                        o = rel + 2 * t2 * ns
                        nc.tensor.matmul(
                            psA[:, :ns],
                            attStat[:, 256 * t2:256 * (t2 + 1)]
                            .rearrange("p (i c) -> p i c", i=2),
                            att_t[:, h, o:o + 2 * ns]
                            .rearrange("p (i n) -> p i n", i=2),
                            start=(t2 == 0), stop=(t2 == 1),
                            perf_mode=DR)
                else:
                    for g in range(4):
                        nc.tensor.matmul(
                            psA[32 * g:32 * (g + 1), :ns], hinF[:],
                            att_t[:, h, rel + g * ns:rel + (g + 1) * ns],
                            start=True, stop=True,
                            tile_position=(0, 32 * g))
                nc.tensor.matmul(psG[:, :ns], lgen2[:, h * 128:(h + 1) * 128],
                                 gen8[:, h, q0:q0 + ns],
                                 start=True, stop=False)
                nc.tensor.matmul(psG[:, :ns], S4all[:, h * 128:(h + 1) * 128],
                                 genb8[:, h, q0:q0 + ns],
                                 start=False, stop=True)
                if ABLATE == "pe":
                    continue
                imp = impp.tile([128, NSUB], F32, tag="imp")
                nc.scalar.activation(imp[:, :ns], psA[:, :ns], AF.Sigmoid,
                                     scale=float(inv_scale))
                tmp = tmpp.tile([128, NSUB], BF16, tag="tmp")
                nc.vector.scalar_tensor_tensor(tmp[:, :ns], psG[:, :ns],
                                               float(gsc), imp[:, :ns],
                                               ALU.mult, ALU.mult)
                tmps.append(tmp)
            if ABLATE != "full":
                continue
            if prev is not None:
                emit_idsum(*prev)
            prev = (s, tmps)
    if prev is not None:
        emit_idsum(*prev)
    else:
        nc.vector.memset(accout[:], 0.0)

    # ---- output writeback (one DMA per supertile) -------------------------
    for s in range(N_SUP + 1):
        ncols = SUP if s < N_SUP else NSUB
        ns = ncols // 4
        nc.scalar.dma_start(
            out[:, s * SUP:s * SUP + ncols].rearrange("p (g c) -> g p c", g=4),
            accout[:, s * NSUB:s * NSUB + ns])
    return nc


_NC_CACHE = None


def _get_nc():
    global _NC_CACHE
    if _NC_CACHE is None:
        _NC_CACHE = _build_bass()
    return _NC_CACHE


# ---------------------------------------------------------------------------
# Host wrapper
# ---------------------------------------------------------------------------
LAST_RESULTS = None
LAST_IN_MAPS = None


def kernel(x, fe_W1, fe_b1, fe_W2, fe_b2, embeds,
           gen_W1, gen_b1, gen_W2, gen_b2, att_W, att_b,
           gate_W, gate_b):
    f32 = np.float32
    np_f8 = mybir.dt.np(F8)
    np_bf = mybir.dt.np(BF16)
    x = np.asarray(x, f32)
    fe_W1 = np.asarray(fe_W1, f32)
    fe_b1 = np.asarray(fe_b1, f32)
    fe_W2 = np.asarray(fe_W2, f32)
    fe_b2 = np.asarray(fe_b2, f32)
    embeds = np.asarray(embeds, f32)
    gen_W1 = np.asarray(gen_W1, f32)
    gen_b1 = np.asarray(gen_b1, f32)
    gen_W2 = np.asarray(gen_W2, f32)
    gen_b2 = np.asarray(gen_b2, f32)
    att_W = np.asarray(att_W, f32)
    att_b = np.asarray(att_b, f32)
    gate_W = np.asarray(gate_W, f32)
    gate_b = np.asarray(gate_b, f32)

    # --- att stream: [H, 68, T_pad] (feats rows + rank-4 embeds/bias fold) --
    tpad = NCORES * TS
    att_all = np.zeros((H, KA, tpad), np_f8)
    att_all[:, :FEAT, :T] = (att_W[:, :, :FEAT].transpose(0, 2, 1)
                             * SC_W).astype(np_f8)
    F = np.einsum('pj,htj->hpt', embeds, att_W[:, :, FEAT:]) \
        + att_b[:, None, :]
    att_all[:, FEAT:, :T] = (F * SC_W).astype(np_f8)
    # --- gen stream ---------------------------------------------------------
    G8 = np.zeros((H, GH, tpad), np.float32)
    G8[:, :, :T] = gen_W2.transpose(0, 2, 1) * SC_W
    genb_all = np.zeros((H, 1, tpad), np_bf)
    genb_all[:, 0, :T] = gen_b2.astype(np_bf)

    # --- const blobs ---
    xt = np.zeros((KFE, B), f32)
    xt[:784] = x.T
    fe1t = np.zeros((KFE, 128), f32)
    fe1t[:784] = fe_W1.T
    gwt = np.concatenate([gate_W.T, gate_b[None, :]], axis=0)
    emb = np.repeat(embeds.T[:, :, None], B, axis=2).reshape(EMB, PB)
    sel4 = np.tile(np.eye(B, dtype=f32), NP)
    g1in = np.concatenate([gen_W1.transpose(0, 2, 1), gen_b1[:, None, :]],
                          axis=1)                      # [H, 97, 32]
    g1in = g1in.transpose(1, 0, 2).reshape(HIN + 1, H * GH)

    def pad128(a):
        o = np.zeros((128, a.shape[1]), f32)
        o[:a.shape[0]] = a
        return o

    cb = np.zeros((128, CB_TOT), f32)
    cb[:, CB_FE1:CB_XT] = fe1t.reshape(7, 128, 128).transpose(1, 0, 2) \
        .reshape(128, 896)
    cb[:, CB_XT:CB_FW2] = xt.reshape(7, 128, B).transpose(1, 0, 2) \
        .reshape(128, 56)
    cb[:, CB_FW2:CB_GWT] = fe_W2.T
    cb[:, CB_GWT:CB_SEL4] = pad128(gwt)
    cb[:, CB_SEL4:CB_G1] = pad128(sel4)
    cb[:, CB_G1:CB_ID] = pad128(g1in)
    cb[:, CB_ID:CB_EMB] = np.eye(128, dtype=f32)
    cb[:, CB_EMB:CB_TOT] = pad128(emb)

    fblob_a = np.zeros((128, 2), f32)
    fblob_a[:, 0] = fe_b1
    fblob_a[:FEAT, 1] = fe_b2

    # onehot rows for the DoubleRow att stationary: [4, 512]; subtile i of
    # tile t carries the onehot block at columns 256t+128i+32(2t+i)
    oh = np.repeat(np.eye(NP, dtype=f32), B, axis=1) * SC_OH   # [4, 32]
    ohdr_a = np.zeros((NP, 512), f32)
    for t2 in range(2):
        for i in range(2):
            g = 2 * t2 + i
            o = 256 * t2 + 128 * i + 32 * g
            ohdr_a[:, o:o + PB] = oh

    shared = {
        "cblob": np.ascontiguousarray(cb.astype(np_bf)),
        "fblob": fblob_a,
        "ohdr": np.ascontiguousarray(ohdr_a.astype(np_f8)),
    }

    def _regroup(a):
        # [H, k, TS] -> [H, 4k, TS//4]: column group g of each supertile
        # becomes contraction rows [g*k, (g+1)*k)
        k = a.shape[1]
        o = np.zeros((H, 4 * k, TS // 4), a.dtype)
        for s in range(7):
            c0 = s * SUP
            ns = (SUP if s < 6 else NSUB) // 4
            for g in range(4):
                o[:, g * k:(g + 1) * k, s * NSUB:s * NSUB + ns] = \
                    a[:, :, c0 + g * ns:c0 + (g + 1) * ns]
        return o

    in_maps = []
    for c in range(NCORES):
        sl = slice(c * TS, (c + 1) * TS)
        m = dict(shared)
        m["att_in"] = np.ascontiguousarray(att_all[:, :, sl])
        m["gen_in"] = np.ascontiguousarray(
            _regroup(G8[:, :, sl]).astype(np_f8))
        m["genb_in"] = np.ascontiguousarray(_regroup(genb_all[:, :, sl]))
        in_maps.append(m)

    nc = _get_nc()
    res = run_bass_kernel_spmd(nc, in_maps, core_ids=list(range(NCORES)))
    global LAST_RESULTS, LAST_IN_MAPS
    LAST_RESULTS = res
    LAST_IN_MAPS = in_maps

    full = np.concatenate(
        [np.asarray(res.results[c]["out"], np.float32) for c in range(NCORES)],
        axis=1)[:, :T]                                # [32, T], row = p*8+b
    return np.ascontiguousarray(
        full.reshape(NP, B, T).transpose(1, 0, 2).reshape(B, NP * T))


# ---------------------------------------------------------------------------
# Timing harness (test-only): device-resident inputs, repeated execution.
# ---------------------------------------------------------------------------
def benchmark_last(in_maps, iters=8, nc=None):
    import time

    import jax
    from concourse import bass2jax as b2j
    from concourse import mybir as _mybir

    if nc is None:
        nc = _get_nc()
    b2j.install_neuronx_cc_hook()

    partition_name = (nc.partition_id_tensor.name
                      if nc.partition_id_tensor else None)
    in_names, out_names, out_avals, zero_outs = [], [], [], []
    for alloc in nc.m.functions[0].allocations:
        if not isinstance(alloc, _mybir.MemoryLocationSet):
            continue
        name = alloc.memorylocations[0].name
        if alloc.kind == "ExternalInput":
            if name != partition_name:
                in_names.append(name)
        elif alloc.kind == "ExternalOutput":
            shape = tuple(alloc.tensor_shape)
            dtype = _mybir.dt.np(alloc.dtype)
            out_names.append(name)
            out_avals.append(jax.core.ShapedArray(shape, dtype))
            zero_outs.append(np.zeros(shape, dtype))
    n_params = len(in_names)
    n_outs = len(out_avals)
    in_names_all = in_names + out_names
    if partition_name is not None:
        in_names_all.append(partition_name)

    def _body(*args):
        operands = list(args)
        if partition_name is not None:
            operands.append(b2j.partition_id_tensor())
        return tuple(b2j._bass_exec_p.bind(
            *operands,
            out_avals=tuple(out_avals),
            in_names=tuple(in_names_all),
            out_names=tuple(out_names),
            lowering_input_output_aliases=(),
            sim_require_finite=True,
            sim_require_nnan=True,
            nc=nc,
        ))

    donate = tuple(range(n_params, n_params + n_outs))
    devices = jax.devices()[:NCORES]
    mesh = b2j.Mesh(np.asarray(devices), ("core",))
    sharded = jax.jit(
        b2j.shard_map(_body, mesh=mesh,
                      in_specs=(b2j.PartitionSpec("core"),) * (n_params + n_outs),
                      out_specs=(b2j.PartitionSpec("core"),) * n_outs,
                      check_rep=False),
        donate_argnums=donate, keep_unused=True)

    concat_in = [
        np.concatenate([np.asarray(in_maps[c][nm]) for c in range(NCORES)],
                       axis=0)
        for nm in in_names
    ]
    sharding = jax.sharding.NamedSharding(mesh, b2j.PartitionSpec("core"))
    dev_in = [jax.device_put(a, sharding) for a in concat_in]

    def _zeros():
        return [jax.device_put(
            np.zeros((NCORES * z.shape[0], *z.shape[1:]), z.dtype), sharding)
            for z in zero_outs]

    # warmup (compile + load)
    outs = sharded(*dev_in, *_zeros())
    jax.block_until_ready(outs)
    times = []
    for _ in range(iters):
        zs = _zeros()
        jax.block_until_ready(zs)
        t0 = time.perf_counter()
        outs = sharded(*dev_in, *zs)
        jax.block_until_ready(outs)
        times.append(time.perf_counter() - t0)
    return min(times), times
